# revision 7
# baseline (speedup 1.0000x reference)
"""Trainium2 Bass kernel for nn_AutodiffChannel: 6-biquad EQ cascade over
(64, 1, 262144) fp32 audio, data-parallel over 8 NeuronCores.

Algorithm (per sequence, LTI block-state decomposition):
  The 6-stage DF2T biquad cascade is a 12-state linear system
  s' = A s + B x, y = C s + D x.  Split T=262144 into 2048 chunks of
  L=128.  Then per chunk c:
      y_c = Phi x_c + Gamma S_c          (Phi  = 128x128 lower-tri Toeplitz
                                          of the impulse response h[0:128],
                                          Gamma[m,:] = C A^m)
      U_c = M x_c                        (M[:,n] = A^(127-n) B)
      S_c = sum_{j<c} (A^128)^(c-1-j) U_j   (exclusive prefix "state scan")
  The prefix is computed with a Kogge-Stone scan (11 levels) using
  precomputed powers P_d = (A^128)^(2^d).  All the O(T) work (Phi/M/Gamma
  matmuls + scan) runs on the PE/DVE/ACT engines; the tiny per-sequence
  setup (h, Gamma, M, P_d: ~0.1% of total FLOPs, independent of T) is
  computed host-side in float64 from the fp32-quantized biquad coeffs.

Device dataflow per core (8 sequences):
  natural x (128p x 2048f per seq) --PE transpose--> XT (chunk columns,
  digit-reversed order c = p*16+j) --> U matmuls --> scan on a (96 x 2049)
  state buffer --> per chunk-group FIR+correction matmuls with the x-block
  as the *stationary* operand, which yields y directly back in natural
  layout --> DMA out.
"""
import sys

for _p in ("/opt/trn_rl_repo", "/opt/trn_rl_repo/concourse"):
    if _p not in sys.path:
        sys.path.insert(0, _p)

import numpy as np

import concourse.bacc as bacc
import concourse.mybir as mybir
from concourse.tile import TileContext
from concourse.bass_utils import run_bass_kernel_spmd

# ---------------------------------------------------------------- problem dims
B, C, T = 64, 1, 262144
N_CORES = 8
SEQ_PER_CORE = B * C // N_CORES  # 8
L = 128                     # chunk length
NCH = T // L                # 2048 chunks per sequence
ROWS = 128                  # natural-layout partitions per sequence
COLS = T // ROWS            # 2048
JG = COLS // L              # 16 chunk-interleave factor (c = p*16 + j)
LEVELS = 11                 # ceil(log2(NCH))
NSTATE = 12
F32 = mybir.dt.float32

PARAM_RANGES = np.array([
    [-24.0, 24.0], [20.0, 200.0], [0.1, 10.0],
    [-24.0, 24.0], [200.0, 2000.0], [0.1, 10.0],
    [-24.0, 24.0], [200.0, 2000.0], [0.1, 10.0],
    [-24.0, 24.0], [2000.0, 8000.0], [0.1, 10.0],
    [-24.0, 24.0], [4000.0, 12000.0], [0.1, 10.0],
    [-24.0, 24.0], [4000.0, 12000.0], [0.1, 10.0],
], dtype=np.float32)
FILTER_TYPES = ["low_shelf", "peaking", "peaking", "peaking", "peaking",
                "high_shelf"]


# ------------------------------------------------------------- host-side setup
def _sigmoid_f32(z):
    z = z.astype(np.float32)
    out = np.empty_like(z)
    pos = z >= 0
    out[pos] = (np.float32(1.0) / (np.float32(1.0) + np.exp(-z[pos]))).astype(
        np.float32)
    ez = np.exp(z[~pos]).astype(np.float32)
    out[~pos] = (ez / (np.float32(1.0) + ez)).astype(np.float32)
    return out


def _biquad_coeffs_f32(g, f, q, sr, ftype):
    """fp32-faithful audio-EQ-cookbook coefficients (matches reference)."""
    f32 = np.float32
    A = np.power(f32(10.0), (g / f32(40.0)).astype(f32)).astype(f32)
    w0 = (f32(2.0) * f32(np.pi) * (f / f32(sr))).astype(f32)
    alpha = (np.sin(w0, dtype=f32) / (f32(2.0) * q)).astype(f32)
    c = np.cos(w0, dtype=f32)
    sA = np.sqrt(A).astype(f32)
    one, two = f32(1.0), f32(2.0)
    if ftype == "low_shelf":
        b0 = A * ((A + one) - (A - one) * c + two * sA * alpha)
        b1 = two * A * ((A - one) - (A + one) * c)
        b2 = A * ((A + one) - (A - one) * c - two * sA * alpha)
        a0 = (A + one) + (A - one) * c + two * sA * alpha
        a1 = -two * ((A - one) + (A + one) * c)
        a2 = (A + one) + (A - one) * c - two * sA * alpha
    elif ftype == "high_shelf":
        b0 = A * ((A + one) + (A - one) * c + two * sA * alpha)
        b1 = -two * A * ((A - one) + (A + one) * c)
        b2 = A * ((A + one) + (A - one) * c - two * sA * alpha)
        a0 = (A + one) - (A - one) * c + two * sA * alpha
        a1 = two * ((A - one) - (A + one) * c)
        a2 = (A + one) - (A - one) * c - two * sA * alpha
    else:
        b0 = one + alpha * A
        b1 = -two * c
        b2 = one - alpha * A
        a0 = one + alpha / A
        a1 = -two * c
        a2 = one - alpha / A
    bc = (np.stack([b0, b1, b2], -1).astype(f32) / a0[..., None]).astype(f32)
    ac = (np.stack([a0, a1, a2], -1).astype(f32) / a0[..., None]).astype(f32)
    return bc, ac


def _coeffs_from_inputs(p, W, b, sample_rate):
    z = (p.astype(np.float32) @ W.astype(np.float32).T
         + b.astype(np.float32)).astype(np.float32)
    pn = _sigmoid_f32(z)
    lo, hi = PARAM_RANGES[:, 0], PARAM_RANGES[:, 1]
    params = (pn * (hi - lo) + lo).astype(np.float32)
    bcs, acs = [], []
    for k, ftype in enumerate(FILTER_TYPES):
        bc, ac = _biquad_coeffs_f32(
            params[:, 3 * k], params[:, 3 * k + 1], params[:, 3 * k + 2],
            float(sample_rate), ftype)
        bcs.append(bc)
        acs.append(ac)
    return np.stack(bcs), np.stack(acs)  # (6, B, 3) fp32


def _state_space(bc, ac):
    """Vectorized float64 (A, B, C, D) per sequence from fp32 DF2T coeffs."""
    nb = bc.shape[1]
    bc64 = bc.astype(np.float64)
    ac64 = ac.astype(np.float64)

    def step(s, x):
        # s: (nb, 12); x: (nb,) -> s', y
        s = s.copy()
        v = x
        for k in range(6):
            b0, b1, b2 = bc64[k, :, 0], bc64[k, :, 1], bc64[k, :, 2]
            a1, a2 = ac64[k, :, 1], ac64[k, :, 2]
            s1, s2 = s[:, 2 * k], s[:, 2 * k + 1]
            y = b0 * v + s1
            s[:, 2 * k] = b1 * v - a1 * y + s2
            s[:, 2 * k + 1] = b2 * v - a2 * y
            v = y
        return s, v

    A = np.zeros((nb, NSTATE, NSTATE))
    Cv = np.zeros((nb, NSTATE))
    for i in range(NSTATE):
        e = np.zeros((nb, NSTATE))
        e[:, i] = 1.0
        sp, y = step(e, np.zeros(nb))
        A[:, :, i] = sp
        Cv[:, i] = y
    Bv, D = step(np.zeros((nb, NSTATE)), np.ones(nb))
    return A, Bv, Cv, D


def _derived(A, Bv, Cv, D):
    """h (nb,L), Gamma (nb,L,12), M (nb,12,L), Pd (nb,LEVELS,12,12) in f64."""
    nb = A.shape[0]
    h = np.zeros((nb, L))
    Gam = np.zeros((nb, L, NSTATE))
    M = np.zeros((nb, NSTATE, L))
    h[:, 0] = D
    cam = Cv.copy()          # C A^m
    amb = Bv.copy()          # A^m B
    for m in range(L):
        Gam[:, m, :] = cam
        M[:, :, L - 1 - m] = amb
        if m + 1 < L:
            h[:, m + 1] = np.einsum("bi,bi->b", cam, Bv)
        cam = np.einsum("bi,bij->bj", cam, A)
        amb = np.einsum("bij,bj->bi", A, amb)
    sq = A.copy()
    for _ in range(7):       # A^(2^7) = A^128
        sq = sq @ sq
    Pd = np.zeros((nb, LEVELS, NSTATE, NSTATE))
    for d in range(LEVELS):
        Pd[:, d] = sq
        sq = sq @ sq
    return h, Gam, M, Pd


def _pack_weights(h, Gam, M, Pd):
    """fp32 device weight tensors, per core."""
    nb = h.shape[0]
    m_idx = np.arange(L)
    diff = m_idx[None, :] - m_idx[:, None]          # [n, m] = m - n
    toepT = np.where(diff >= 0, h[:, np.clip(diff, 0, L - 1)],
                     0.0).astype(np.float32)        # (nb, n=128, m=128)
    # embedded at per-seq 12-row offsets inside a 96-row frame so every
    # device access stays at base partition 0 (HW requires 32-aligned bases)
    gammaT = np.zeros((nb, 96, L), np.float32)      # (nb, k-embed, m)
    mT = np.zeros((nb, L, 96), np.float32)          # (nb, n, k-embed)
    for g in range(nb):
        s8 = g % SEQ_PER_CORE
        gammaT[g, 12 * s8:12 * s8 + 12, :] = Gam[g].T.astype(np.float32)
        mT[g, :, 12 * s8:12 * s8 + 12] = M[g].T.astype(np.float32)
    scanP = np.zeros((N_CORES, LEVELS, 96, 96), np.float32)
    for core in range(N_CORES):
        for s in range(SEQ_PER_CORE):
            g = core * SEQ_PER_CORE + s
            for d in range(LEVELS):
                scanP[core, d, 12 * s:12 * s + 12, 12 * s:12 * s + 12] = \
                    Pd[g, d].T.astype(np.float32)
    return toepT, gammaT, mT, scanP


# ------------------------------------------------------------ device kernel IR
_NC_CACHE = {}


def build_nc(rep=1):
    if rep in _NC_CACHE:
        return _NC_CACHE[rep]
    nc = bacc.Bacc("TRN2")
    x_d = nc.dram_tensor("x", [SEQ_PER_CORE, ROWS, COLS], F32,
                         kind="ExternalInput")
    toepT_d = nc.dram_tensor("toepT", [SEQ_PER_CORE, L, L], F32,
                             kind="ExternalInput")
    gammaT_d = nc.dram_tensor("gammaT", [SEQ_PER_CORE, 96, L], F32,
                              kind="ExternalInput")
    mT_d = nc.dram_tensor("mT", [SEQ_PER_CORE, L, 96], F32,
                          kind="ExternalInput")
    scanP_d = nc.dram_tensor("scanP", [LEVELS, 96, 96], F32,
                             kind="ExternalInput")
    ident_d = nc.dram_tensor("ident", [128, 128], F32, kind="ExternalInput")
    y_d = nc.dram_tensor("y", [SEQ_PER_CORE, ROWS, COLS], F32,
                         kind="ExternalOutput")

    with TileContext(nc) as tc:
        with tc.tile_pool(name="weights", bufs=1) as wpool:
            toepT_sb = wpool.tile([L, SEQ_PER_CORE * L], F32)
            nc.sync.dma_start(
                out=toepT_sb[:].rearrange("p (s m) -> p s m", m=L),
                in_=toepT_d[:].transpose([1, 0, 2]))
            gammaT_sb = wpool.tile([96, SEQ_PER_CORE * L], F32)
            nc.sync.dma_start(
                out=gammaT_sb[:].rearrange("k (s m) -> k s m", m=L),
                in_=gammaT_d[:].transpose([1, 0, 2]))
            mT_sb = wpool.tile([L, SEQ_PER_CORE * 96], F32)
            nc.sync.dma_start(
                out=mT_sb[:].rearrange("n (s k) -> n s k", k=96),
                in_=mT_d[:].transpose([1, 0, 2]))
            scanP_sb = wpool.tile([96, LEVELS * 96], F32)
            nc.sync.dma_start(
                out=scanP_sb[:].rearrange("j (d k) -> j d k", k=96),
                in_=scanP_d[:].transpose([1, 0, 2]))
            ident_sb = wpool.tile([128, 128], F32)
            nc.sync.dma_start(out=ident_sb, in_=ident_d[:])

            with tc.tile_pool(name="xt", bufs=1) as xtpool, \
                 tc.tile_pool(name="xn", bufs=2) as xnpool, \
                 tc.tile_pool(name="ysb", bufs=2) as ypool:
                for _ in range(rep):
                    _one_pass(nc, tc, x_d, y_d, toepT_sb, gammaT_sb, mT_sb,
                              scanP_sb, ident_sb, xtpool, xnpool, ypool)
    nc.compile()
    _NC_CACHE[rep] = nc
    return nc


def _one_pass(nc, tc, x_d, y_d, toepT_sb, gammaT_sb, mT_sb, scanP_sb,
              ident_sb, xtpool, xnpool, ypool):
    XT = [xtpool.tile([ROWS, COLS], F32, tag=f"xt{s}", name=f"xt{s}")
          for s in range(SEQ_PER_CORE)]
    with tc.tile_pool(name="wbuf", bufs=1) as wbpool:
        # state buffer: col 0 = zeros, col 1+c = inclusive prefix W_c of
        # chunk c; rows 12s..12s+12 = seq s (all accesses use base 0)
        wb = wbpool.tile([96, NCH + 1], F32, tag="wb")
        nc.gpsimd.memset(wb[:, 0:1], 0.0)
        uview = (wb[0:96, 1:NCH + 1]
                 .rearrange("r (p j) -> r p j", j=JG)
                 .transpose([0, 2, 1]))              # (96, j=16, p=128)

        # ---- phase A: load, transpose to chunk columns, U = M x ----
        with tc.tile_pool(name="tp", bufs=2, space="PSUM") as tpsum, \
             tc.tile_pool(name="up", bufs=2, space="PSUM") as upsum:
            for s in range(SEQ_PER_CORE):
                xn = xnpool.tile([ROWS, COLS], F32, tag="xn")
                nc.sync.dma_start(out=xn, in_=x_d[s])
                for g in range(4):
                    pt = tpsum.tile([128, 512], F32, tag="tp")
                    for jj in range(4):
                        j = 4 * g + jj
                        nc.tensor.transpose(pt[:, jj * 128:(jj + 1) * 128],
                                            xn[:, j * 128:(j + 1) * 128],
                                            ident_sb)
                    nc.scalar.copy(XT[s][:, g * 512:(g + 1) * 512], pt)
            # all 8 seqs accumulate into one 96-row U tile per column block
            for i in range(4):
                up = upsum.tile([96, 512], F32, tag="up")
                for s in range(SEQ_PER_CORE):
                    nc.tensor.matmul(
                        up[:],
                        lhsT=mT_sb[:, s * 96:(s + 1) * 96],
                        rhs=XT[s][:, i * 512:(i + 1) * 512],
                        start=(s == 0), stop=(s == SEQ_PER_CORE - 1))
                nc.vector.tensor_copy(
                    out=uview[:, 4 * i:4 * i + 4, :],
                    in_=up[:].rearrange("r (a b) -> r a b", b=128))

        # ---- state scan: W_c += P_d W_{c-2^d} ----
        with tc.tile_pool(name="sp", bufs=2, space="PSUM") as spsum:
            for d in range(LEVELS):
                sh = 1 << d
                sp = spsum.tile([96, NCH], F32, tag="sp")
                c0 = sh
                while c0 < NCH:
                    c1 = min((c0 // 512 + 1) * 512, NCH)
                    nc.tensor.matmul(sp[:, c0:c1],
                                     lhsT=scanP_sb[:, d * 96:(d + 1) * 96],
                                     rhs=wb[:, 1 + c0 - sh:1 + c1 - sh],
                                     start=True, stop=True)
                    c0 = c1
                nc.vector.tensor_add(out=wb[:, 1 + sh:NCH + 1],
                                     in0=wb[:, 1 + sh:NCH + 1],
                                     in1=sp[:, sh:NCH])

        # ---- phase B: y = Phi x + Gamma S, emitted in natural layout ----
        sview = (wb[0:96, 0:NCH]
                 .rearrange("r (p j) -> r p j", j=JG)
                 .transpose([0, 2, 1]))              # (96, j, p): S_{c(p,j)}
        with tc.tile_pool(name="yp", bufs=4, space="PSUM") as ypsum:
            for s in range(SEQ_PER_CORE):
                ysb = ypool.tile([ROWS, COLS], F32, tag="ysb")
                for g in range(4):
                    yp = ypsum.tile([128, 512], F32, tag="yp")
                    for jj in range(4):
                        j = 4 * g + jj
                        nc.tensor.matmul(
                            yp[:, jj * 128:(jj + 1) * 128],
                            lhsT=XT[s][:, j * 128:(j + 1) * 128],
                            rhs=toepT_sb[:, s * L:(s + 1) * L],
                            start=True, stop=False)
                        nc.tensor.matmul(
                            yp[:, jj * 128:(jj + 1) * 128],
                            lhsT=sview[:, j, :],
                            rhs=gammaT_sb[:, s * L:(s + 1) * L],
                            start=False, stop=True)
                    if g % 2 == 0:
                        nc.scalar.copy(ysb[:, g * 512:(g + 1) * 512], yp)
                    else:
                        nc.vector.tensor_copy(
                            out=ysb[:, g * 512:(g + 1) * 512], in_=yp[:])
                nc.sync.dma_start(out=y_d[s], in_=ysb)


# ----------------------------------------------------------------- entry point
def _prepare_in_maps(x, p, W, b, sample_rate):
    bc, ac = _coeffs_from_inputs(p, W, b, sample_rate)
    A, Bv, Cv, D = _state_space(bc, ac)
    h, Gam, M, Pd = _derived(A, Bv, Cv, D)
    toepT, gammaT, mT, scanP = _pack_weights(h, Gam, M, Pd)
    ident = np.eye(128, dtype=np.float32)
    xs = np.ascontiguousarray(x.reshape(B * C, T).astype(np.float32))
    in_maps = []
    for core in range(N_CORES):
        sl = slice(core * SEQ_PER_CORE, (core + 1) * SEQ_PER_CORE)
        in_maps.append({
            "x": np.ascontiguousarray(
                xs[sl].reshape(SEQ_PER_CORE, ROWS, COLS)),
            "toepT": np.ascontiguousarray(toepT[sl]),
            "gammaT": np.ascontiguousarray(gammaT[sl]),
            "mT": np.ascontiguousarray(mT[sl]),
            "scanP": np.ascontiguousarray(scanP[core]),
            "ident": ident,
        })
    return in_maps


def kernel(x, p, W, b, sample_rate):
    nc = build_nc(rep=1)
    in_maps = _prepare_in_maps(x, p, W, b, sample_rate)
    res = run_bass_kernel_spmd(nc, in_maps, core_ids=list(range(N_CORES)))
    y = np.concatenate(
        [res.results[c]["y"].reshape(SEQ_PER_CORE, T) for c in range(N_CORES)],
        axis=0)
    return y.reshape(B, C, T).astype(np.float32)
